# revision 17
# baseline (speedup 1.0000x reference)
"""Trainium2 Bass kernel for nn_KineticEquation (gnn_message_passing).

Reference computation:
    contrib_1 = y[:, i1r] * rate1                 # [B, R1]
    contrib_2 = y[:, i2r0] * y[:, i2r1] * rate2   # [B, R2]
    y_out = scatter_add(contrib_1 -> i1p) + scatter_add(contrib_2 -> i2p)

Strategy (8 NeuronCores, full batch per core, reactions sharded by product
tile p//128 so each core owns one 128-row slice of y_out^T):

Square-trick formulation of the second-order term:
    rate*y0*y1 = (rate/2)*(y0+y1)^2 - (rate/2)*(y0^2 + y1^2)
The pairwise part becomes, per chunk of 128 reactions sharing a species-tile
pair (T0, T1):
    u   = G0^T @ yT[T0] + G1^T @ yT[T1]   (PE, one-hot lhsT with sqrt(rate/2)
                                           folded in; accumulated in PSUM)
    u2  = Square(u)                        (ACT or DVE, PSUM -> SBUF fp16)
    acc += SC^T @ u2                       (PE, 0/1 scatter into the product
                                           tile accumulator)
The -(rate/2)(y0^2+y1^2) part is a *dense* per-species correction: it only
depends on (species, product) marginals, so it folds into a dense matrix W2
applied to w = y^2, merged with the first-order dense term W1 applied to y:
    acc += W1g^T @ yT  (8 matmuls)  +  W2g^T @ wT  (8 matmuls)
with w computed on-device by DVE (y*y, fp16, 2x mode).

This removes the per-chunk ACT copy + DVE tensor_tensor of the direct
formulation; the only per-chunk non-PE work is one grouped Square.
All per-chunk tensors are fp16 (rel err ~5e-4 after averaging, vs the 2e-2
tolerance); PSUM accumulation is fp32.
"""

import math

import numpy as np

import concourse.tile as tile
from concourse import bacc, mybir
from concourse.bass_utils import run_bass_kernel_spmd

F32 = mybir.dt.float32
F16 = mybir.dt.float16

NCORES = 8
P = 128           # partitions / tile edge
S = 1024          # species
NT = S // P       # species tiles (8)
B = 512           # batch
GRP = 4           # chunks per DMA group
GACT = 3          # chunks per Square group (3 PSUM banks)


def _balance_tiles(i2r0, i2r1, core_of, seconds=45.0, seed=0):
    """Simulated annealing on the species->tile assignment to minimize the
    total chunk count sum_bins ceil(max_core count / 128) over canonical
    (unordered) tile-pair bins."""
    import time as _time
    rng = np.random.default_rng(seed)
    tile = (np.arange(S) // P).astype(np.int64)
    adj = [[] for _ in range(S)]
    for i, (a, b) in enumerate(zip(i2r0, i2r1)):
        adj[a].append(i)
        if b != a:
            adj[b].append(i)
    adj = [np.array(x, np.int64) for x in adj]

    def full_M(tile):
        a = tile[i2r0]
        b = tile[i2r1]
        binid = np.minimum(a, b) * NT + np.maximum(a, b)
        M = np.zeros((NCORES, NT * NT), np.int64)
        for c in range(NCORES):
            M[c] = np.bincount(binid[core_of == c], minlength=NT * NT)
        return M

    M = full_M(tile)

    # chunk cost in matmuls: diagonal bins need 1 gather + 1 scatter,
    # off-diagonal need 2 gathers + 1 scatter
    wbin = np.array([2.0 if (b >> 3) == (b & 7) else 3.0
                     for b in range(NT * NT)])

    def obj(M):
        mx = M.max(axis=0)
        ch = np.ceil(mx / P)
        frac = np.where(mx > 0, (mx - 1) % P + 1, 0) / P
        return float((ch * wbin).sum() + 0.02 * frac.sum())

    cur = obj(M)
    Temp = 1.0
    t0 = _time.time()
    it = 0
    while _time.time() - t0 < seconds:
        it += 1
        s1, s2 = rng.integers(0, S, 2)
        if tile[s1] == tile[s2]:
            continue
        affected = np.unique(np.concatenate([adj[s1], adj[s2]]))
        old1, old2 = tile[s1], tile[s2]
        a = tile[i2r0[affected]]
        b = tile[i2r1[affected]]
        oldbins = np.minimum(a, b) * NT + np.maximum(a, b)
        tile[s1], tile[s2] = old2, old1
        a = tile[i2r0[affected]]
        b = tile[i2r1[affected]]
        newbins = np.minimum(a, b) * NT + np.maximum(a, b)
        cc = core_of[affected]
        np.subtract.at(M, (cc, oldbins), 1)
        np.add.at(M, (cc, newbins), 1)
        new = obj(M)
        if new <= cur or rng.random() < np.exp(-(new - cur) / Temp):
            cur = new
        else:
            np.subtract.at(M, (cc, newbins), 1)
            np.add.at(M, (cc, oldbins), 1)
            tile[s1], tile[s2] = old1, old2
        if it % 3000 == 0:
            Temp *= 0.75
    return tile


def _preprocess(y_in, i1r, i1p, r1, i2r0, i2r1, i2p, r2, sa_seconds=45.0):
    """Host-side index preprocessing. Returns per-core input dicts + schedule."""
    i1r = np.asarray(i1r).astype(np.int64)
    i1p = np.asarray(i1p).astype(np.int64)
    i2r0 = np.asarray(i2r0).astype(np.int64)
    i2r1 = np.asarray(i2r1).astype(np.int64)
    i2p = np.asarray(i2p).astype(np.int64)
    r1 = np.asarray(r1).astype(np.float32)
    r2 = np.asarray(r2).astype(np.float32)

    core_of = i2p >> 7

    # Species permutation: SA-balanced tile assignment, then renumber
    # species so each tile is contiguous.
    tile_of = _balance_tiles(i2r0, i2r1, core_of, seconds=sa_seconds)
    order = np.argsort(tile_of, kind="stable")  # new_idx -> orig species
    new_idx = np.empty(S, np.int64)
    new_idx[order] = np.arange(S)

    i1r = new_idx[i1r]
    i2r0 = new_idx[i2r0]
    i2r1 = new_idx[i2r1]

    # Canonicalize second-order pairs so tile(r0) <= tile(r1)
    swap = (i2r0 >> 7) > (i2r1 >> 7)
    i2r0s = np.where(swap, i2r1, i2r0)
    i2r1s = np.where(swap, i2r0, i2r1)
    i2r0, i2r1 = i2r0s, i2r1s

    # Dense first-order matrix W1[s, p] = sum of rates (permuted rows)
    W1 = np.zeros((S, S), np.float32)
    np.add.at(W1, (i1r, i1p), r1)
    # Dense second-order correction W2[s, p] = -sum rate/2 over reactions
    # where s appears as either reactant and p is the product.
    W2 = np.zeros((S, S), np.float32)
    np.add.at(W2, (i2r0, i2p), -0.5 * r2)
    np.add.at(W2, (i2r1, i2p), -0.5 * r2)

    yT = np.ascontiguousarray(np.asarray(y_in, np.float32).T[order])  # [S, B] permuted

    g2 = np.sqrt(0.5 * r2).astype(np.float32)

    T0 = i2r0 >> 7
    T1 = i2r1 >> 7
    binid = (T0 << 3) | T1  # canonical: T0 <= T1

    counts = np.zeros((NCORES, NT * NT), np.int64)
    for c in range(NCORES):
        counts[c] = np.bincount(binid[core_of == c], minlength=NT * NT)
    maxc = counts.max(axis=0)

    # ---- chunk layout ----
    # Off-diagonal bins use fp8 DoubleRow gathers (one [128,2,128] one-hot
    # against a resident hi/lo-split y tile-pair: 2 DR matmuls per span).
    # Diagonal bins use a single fp16 gather matmul. Rates live in SC.
    # Each chunk = list of spans (bin, col0, ncols); remainders of any bins
    # pack together freely (each span brings its own gather matrix).
    full_n = maxc // P
    rem_n = maxc % P
    chunk_descs = []   # list of spans [(bin, col0, ncols)]
    bin_slots = {}     # bin -> list of (chunk_idx, col0) per 128-block
    for b in range(NT * NT):
        if maxc[b] == 0:
            continue
        slots = []
        for _f in range(int(full_n[b])):
            chunk_descs.append([(b, 0, P)])
            slots.append((len(chunk_descs) - 1, 0))
        bin_slots[b] = slots

    # remainders: first-fit-decreasing, no sharing constraint
    rem = sorted(((int(rem_n[b]), b) for b in range(NT * NT) if rem_n[b] > 0),
                 reverse=True)
    open_chunks = []  # (free, chunk_idx)
    for sz, b in rem:
        placed = False
        for oi, (free, ci) in enumerate(open_chunks):
            if sz <= free:
                col0 = P - free
                chunk_descs[ci].append((b, col0, sz))
                bin_slots.setdefault(b, []).append((ci, col0))
                open_chunks[oi] = (free - sz, ci)
                placed = True
                break
        if not placed:
            chunk_descs.append([(b, 0, sz)])
            bin_slots.setdefault(b, []).append((len(chunk_descs) - 1, 0))
            open_chunks.append((P - sz, len(chunk_descs) - 1))

    nchunk = len(chunk_descs)
    ngroup = math.ceil(nchunk / GRP)
    nchpad = ngroup * GRP

    # canonical pairs used by off-diagonal spans -> resident pair tensors
    pair_idx = {}
    # stream assignment: per span either ('dr', pair, m8) or ('f16', tile, m16)
    sched = []
    span_mat = {}      # (chunk, bin) -> ('dr'|'f16', mat index)
    nmat8 = nmat16 = 0
    for ci, spans in enumerate(chunk_descs):
        ents = []
        for (b, col0, ncols) in spans:
            t0, t1 = b >> 3, b & 7
            if t0 == t1:
                ents.append(("f16", t0))
                span_mat[(ci, b)] = ("f16", nmat16)
                nmat16 += 1
            else:
                if (t0, t1) not in pair_idx:
                    pair_idx[(t0, t1)] = len(pair_idx)
                ents.append(("dr", pair_idx[(t0, t1)]))
                span_mat[(ci, b)] = ("dr", nmat8)
                nmat8 += 1
        sched.append(ents)
    npair = len(pair_idx)
    ngm8 = math.ceil(max(nmat8, 1) / GRP)
    ngm16 = math.ceil(max(nmat16, 1) / GRP)

    # fp8 hi/lo split of y (resident pair tensors)
    import ml_dtypes
    f8 = ml_dtypes.float8_e4m3
    y8hi = yT.astype(f8)
    y8lo = (yT - y8hi.astype(np.float32)).astype(f8)
    YPH = np.zeros((npair, P, 2, B), f8)
    YPL = np.zeros((npair, P, 2, B), f8)
    for (a, bb), pi in pair_idx.items():
        YPH[pi, :, 0, :] = y8hi[a * P:(a + 1) * P]
        YPH[pi, :, 1, :] = y8hi[bb * P:(bb + 1) * P]
        YPL[pi, :, 0, :] = y8lo[a * P:(a + 1) * P]
        YPL[pi, :, 1, :] = y8lo[bb * P:(bb + 1) * P]

    halfr = (0.5 * r2).astype(np.float32)

    in_maps = []
    for c in range(NCORES):
        sel = core_of == c
        bsel = binid[sel]
        order = np.argsort(bsel, kind="stable")
        bs = bsel[order]
        r0a = i2r0[sel][order]
        r1a = i2r1[sel][order]
        pl = (i2p[sel] & 127)[order]
        hr = halfr[sel][order]
        bin_start = np.zeros(NT * NT, np.int64)
        cnt = np.bincount(bs, minlength=NT * NT)
        bin_start[1:] = np.cumsum(cnt)[:-1]
        pos = np.arange(len(bs)) - bin_start[bs]

        chunk = np.empty(len(bs), np.int64)
        col = np.empty(len(bs), np.int64)
        for b in np.unique(bs):
            slots = bin_slots[b]
            m = bs == b
            p_ = pos[m]
            blk = (p_ >> 7).astype(np.int64)
            ci = np.array([slots[k][0] for k in blk])
            c0 = np.array([slots[k][1] for k in blk])
            chunk[m] = ci
            col[m] = c0 + (p_ & 127)

        G8 = np.zeros((ngm8 * GRP, P, 2, P), np.float32)
        G16 = np.zeros((ngm16 * GRP, P, P), np.float32)
        SC = np.zeros((nchpad, P, P), np.float32)
        for i in range(len(bs)):
            b = bs[i]
            kind, mi = span_mat[(chunk[i], b)]
            if kind == "dr":
                G8[mi, r0a[i] & 127, 0, col[i]] += 1.0
                G8[mi, r1a[i] & 127, 1, col[i]] += 1.0
            else:
                G16[mi, r0a[i] & 127, col[i]] += 1.0
                G16[mi, r1a[i] & 127, col[i]] += 1.0
        SC[chunk, col, pl] = hr

        g8g = np.ascontiguousarray(
            G8.reshape(ngm8, GRP, P, 2, P).transpose(0, 2, 1, 3, 4).astype(f8))
        g16g = np.ascontiguousarray(
            G16.reshape(ngm16, GRP, P, P).transpose(0, 2, 1, 3)
            .reshape(ngm16, P, GRP * P).astype(np.float16))
        scg = np.ascontiguousarray(
            SC.reshape(ngroup, GRP, P, P).transpose(0, 2, 1, 3)
            .reshape(ngroup, P, GRP * P).astype(np.float16))

        in_maps.append(
            dict(
                yT=yT.astype(np.float16),
                W1g=np.ascontiguousarray(W1[:, c * P:(c + 1) * P]).astype(np.float16),
                W2g=np.ascontiguousarray(W2[:, c * P:(c + 1) * P]).astype(np.float16),
                YPH=YPH, YPL=YPL,
                G8=g8g, G16=g16g, SCT=scg,
            )
        )
    return in_maps, sched, nchunk, (ngroup, ngm8, ngm16, npair)


def _build(nchunk, ngroups, sched, reps=1, bufs_oh=6, bufs_u=2, bufs_u2=4,
           bufs_acc=2, sq_pattern="A"):
    """sq_pattern: cycle of engines for the Square groups ('A'=ACT, 'D'=DVE)."""
    ngroup, ngm8, ngm16, npair = ngroups
    F8 = mybir.dt.float8e4
    DR = mybir.MatmulPerfMode.DoubleRow
    nc = bacc.Bacc("TRN2", target_bir_lowering=False, debug=False, num_devices=NCORES)

    yT_d = nc.dram_tensor("yT", [S, B], F16, kind="ExternalInput").ap()
    w1_d = nc.dram_tensor("W1g", [S, P], F16, kind="ExternalInput").ap()
    w2_d = nc.dram_tensor("W2g", [S, P], F16, kind="ExternalInput").ap()
    yph_d = nc.dram_tensor("YPH", [npair, P, 2, B], F8, kind="ExternalInput").ap()
    ypl_d = nc.dram_tensor("YPL", [npair, P, 2, B], F8, kind="ExternalInput").ap()
    g8_d = nc.dram_tensor("G8", [ngm8, P, GRP, 2, P], F8, kind="ExternalInput").ap()
    g16_d = nc.dram_tensor("G16", [ngm16, P, GRP * P], F16, kind="ExternalInput").ap()
    sc_d = nc.dram_tensor("SCT", [ngroup, P, GRP * P], F16, kind="ExternalInput").ap()
    out_d = nc.dram_tensor("out", [P, B], F32, kind="ExternalOutput").ap()

    nact = math.ceil(nchunk / GACT)

    with tile.TileContext(nc) as tc:
        with (
            tc.tile_pool(name="res", bufs=1) as res,
            tc.tile_pool(name="oh", bufs=bufs_oh) as ohp,
            tc.tile_pool(name="work", bufs=3) as wp,
            tc.tile_pool(name="u2p", bufs=bufs_u2) as u2p,
            tc.tile_pool(name="acc", bufs=bufs_acc, space="PSUM") as accp,
            tc.tile_pool(name="up", bufs=bufs_u, space="PSUM") as upp,
        ):
            # Resident tiles: y^T species tiles, W1/W2 slices, fp8 y pairs
            yts = []
            for t in range(NT):
                yt = res.tile([P, B], F16, tag=f"yt{t}")
                nc.sync.dma_start(yt[:], yT_d[t * P:(t + 1) * P, :])
                yts.append(yt)
            w1t = res.tile([P, NT * P], F16, tag="w1")
            w2t = res.tile([P, NT * P], F16, tag="w2")
            for t in range(NT):
                nc.sync.dma_start(w1t[:, t * P:(t + 1) * P], w1_d[t * P:(t + 1) * P, :])
                nc.sync.dma_start(w2t[:, t * P:(t + 1) * P], w2_d[t * P:(t + 1) * P, :])
            yphs, ypls = [], []
            for pi in range(npair):
                th = res.tile([P, 2, B], mybir.dt.float8e4, tag=f"yph{pi}")
                tl = res.tile([P, 2, B], mybir.dt.float8e4, tag=f"ypl{pi}")
                nc.sync.dma_start(th[:], yph_d[pi])
                nc.sync.dma_start(tl[:], ypl_d[pi])
                yphs.append(th)
                ypls.append(tl)

            def one_pass():
                acc = accp.tile([P, B], F32, space="PSUM", tag="acc")

                # w = y^2 per species tile (DVE, fp16 2x) — needed only by
                # the trailing dense W2 matmuls, so plenty of slack.
                wts = []
                for t in range(NT):
                    w = wp.tile([P, B], F16, tag=f"w{t}")
                    nc.vector.tensor_tensor(out=w[:], in0=yts[t][:], in1=yts[t][:],
                                            op=mybir.AluOpType.mult)
                    wts.append(w)

                # First-order dense matmuls open the accumulation group.
                for t in range(NT):
                    nc.tensor.matmul(acc[:], lhsT=w1t[:, t * P:(t + 1) * P],
                                     rhs=yts[t][:], start=(t == 0), stop=False)

                # Second-order chunks, grouped by GACT for the Square and by
                # GRP for DMA. Scatters lag one Square-group behind gathers.
                from collections import deque
                pendq = deque()  # (u2_tile, [(chunk_k, sc_slice)...]) with lag 2
                gi_cur = g8_cur = g16_cur = -1
                m8pos = m16pos = 0
                g8t = g16t = None
                for ga in range(nact):
                    c0 = ga * GACT
                    cw = min(GACT, nchunk - c0)
                    ug = upp.tile([P, GACT * B], F32, space="PSUM", tag="ug")
                    scs = []
                    pend = pendq.popleft() if len(pendq) >= 2 else None
                    psc = list(pend[1]) if pend is not None else []
                    for k in range(cw):
                        c = c0 + k
                        ents = sched[c]
                        gi = c // GRP
                        if gi != gi_cur:
                            gi_cur = gi
                            scg = ohp.tile([P, GRP * P], F16, tag="scg")
                            nc.sync.dma_start(scg[:], sc_d[gi])
                        us = ug[:, k * B:(k + 1) * B]
                        nmm = sum(2 if e[0] == "dr" else 1 for e in ents)
                        mi = 0
                        for e in ents:
                            if e[0] == "dr":
                                pi = e[1]
                                mg, mk = divmod(m8pos, GRP)
                                if mg != g8_cur:
                                    g8_cur = mg
                                    g8t = ohp.tile([P, GRP, 2, P],
                                                   mybir.dt.float8e4, tag="g8g")
                                    nc.sync.dma_start(g8t[:], g8_d[mg])
                                m8pos += 1
                                lh = g8t[:, mk]
                                nc.tensor.matmul(
                                    us, lhsT=lh, rhs=yphs[pi][:],
                                    start=(mi == 0), stop=(mi == nmm - 1),
                                    perf_mode=mybir.MatmulPerfMode.DoubleRow)
                                mi += 1
                                nc.tensor.matmul(
                                    us, lhsT=lh, rhs=ypls[pi][:],
                                    start=(mi == 0), stop=(mi == nmm - 1),
                                    perf_mode=mybir.MatmulPerfMode.DoubleRow)
                                mi += 1
                            else:
                                t = e[1]
                                mg, mk = divmod(m16pos, GRP)
                                if mg != g16_cur:
                                    g16_cur = mg
                                    g16t = ohp.tile([P, GRP * P], F16, tag="g16g")
                                    nc.sync.dma_start(g16t[:], g16_d[mg])
                                m16pos += 1
                                nc.tensor.matmul(
                                    us, lhsT=g16t[:, mk * P:(mk + 1) * P],
                                    rhs=yts[t][:],
                                    start=(mi == 0), stop=(mi == nmm - 1))
                                mi += 1
                        cs = slice((c % GRP) * P, (c % GRP + 1) * P)
                        scs.append((k, scg[:, cs]))
                        # interleave one pending scatter after each gather pair
                        if psc:
                            pk, pscsl = psc.pop(0)
                            nc.tensor.matmul(acc[:], lhsT=pscsl,
                                             rhs=pend[0][:, pk * B:(pk + 1) * B],
                                             start=False, stop=False)

                    u2 = u2p.tile([P, GACT * B], F16, tag="u2")
                    eng = sq_pattern[ga % len(sq_pattern)]
                    if eng == "A":
                        nc.scalar.activation(u2[:, :cw * B], ug[:, :cw * B],
                                             mybir.ActivationFunctionType.Square)
                    else:
                        nc.vector.tensor_tensor(out=u2[:, :cw * B],
                                                in0=ug[:, :cw * B],
                                                in1=ug[:, :cw * B],
                                                op=mybir.AluOpType.mult)

                    # drain any leftover pending scatters
                    for pk, pscsl in psc:
                        nc.tensor.matmul(acc[:], lhsT=pscsl,
                                         rhs=pend[0][:, pk * B:(pk + 1) * B],
                                         start=False, stop=False)
                    pendq.append((u2, scs))

                while pendq:
                    pu2, pscs = pendq.popleft()
                    for k, scsl in pscs:
                        nc.tensor.matmul(acc[:], lhsT=scsl,
                                         rhs=pu2[:, k * B:(k + 1) * B],
                                         start=False, stop=False)

                # Dense W2 correction on w closes the accumulation group.
                for t in range(NT):
                    nc.tensor.matmul(acc[:], lhsT=w2t[:, t * P:(t + 1) * P],
                                     rhs=wts[t][:], start=False, stop=(t == NT - 1))

                outs = wp.tile([P, B], F32, tag="outs")
                nc.vector.tensor_copy(outs[:], acc[:])
                nc.sync.dma_start(out_d[:], outs[:])

            for _rep in range(reps):
                one_pass()

    nc.compile()
    return nc


def _run(inputs, trace=False):
    in_maps, sched, nchunk, ngroup = _preprocess(
        inputs["y_in"], inputs["inds_1r"], inputs["inds_1p"], inputs["rate_1"],
        inputs["inds_2r0"], inputs["inds_2r1"], inputs["inds_2p"], inputs["rate_2"],
    )
    nc = _build(nchunk, ngroup, sched)
    res = None
    y_out = None
    last_exc = None
    for attempt in range(3):
        try:
            res = run_bass_kernel_spmd(nc, in_maps, list(range(NCORES)), trace=trace)
        except Exception as e:  # transient device wedges (NRT_EXEC_UNIT_...)
            last_exc = e
            import time as _time
            _time.sleep(2.0)
            continue
        y_out = np.empty((B, S), np.float32)
        for c in range(NCORES):
            y_out[:, c * P:(c + 1) * P] = res.results[c]["out"].T
        # guard against silent corruption from a wedged device
        if np.isfinite(y_out).all() and not (y_out == 0).all():
            break
        y_out = None
    if y_out is None:
        if last_exc is not None:
            raise last_exc
        raise RuntimeError("kernel produced non-finite/empty output on all attempts")
    return y_out, res


def kernel(**inputs) -> np.ndarray:
    return _run(inputs, trace=False)[0]


# revision 24
# speedup vs baseline: 1.0010x; 1.0010x over previous
"""Trainium2 Bass kernel for nn_KineticEquation (gnn_message_passing).

Reference computation:
    contrib_1 = y[:, i1r] * rate1                 # [B, R1]
    contrib_2 = y[:, i2r0] * y[:, i2r1] * rate2   # [B, R2]
    y_out = scatter_add(contrib_1 -> i1p) + scatter_add(contrib_2 -> i2p)

Strategy (8 NeuronCores, full batch per core, reactions sharded by product
tile p//128 so each core owns one 128-row slice of y_out^T):

Square-trick formulation of the second-order term:
    rate*y0*y1 = (rate/2)*(y0+y1)^2 - (rate/2)*(y0^2 + y1^2)
The pairwise part becomes, per chunk of 128 reactions sharing a species-tile
pair (T0, T1):
    u   = G0^T @ yT[T0] + G1^T @ yT[T1]   (PE, one-hot lhsT with sqrt(rate/2)
                                           folded in; accumulated in PSUM)
    u2  = Square(u)                        (ACT or DVE, PSUM -> SBUF fp16)
    acc += SC^T @ u2                       (PE, 0/1 scatter into the product
                                           tile accumulator)
The -(rate/2)(y0^2+y1^2) part is a *dense* per-species correction: it only
depends on (species, product) marginals, so it folds into a dense matrix W2
applied to w = y^2, merged with the first-order dense term W1 applied to y:
    acc += W1g^T @ yT  (8 matmuls)  +  W2g^T @ wT  (8 matmuls)
with w computed on-device by DVE (y*y, fp16, 2x mode).

This removes the per-chunk ACT copy + DVE tensor_tensor of the direct
formulation; the only per-chunk non-PE work is one grouped Square.
All per-chunk tensors are fp16 (rel err ~5e-4 after averaging, vs the 2e-2
tolerance); PSUM accumulation is fp32.
"""

import math

import numpy as np

import concourse.tile as tile
from concourse import bacc, mybir
from concourse.bass_utils import run_bass_kernel_spmd

F32 = mybir.dt.float32
F16 = mybir.dt.float16

NCORES = 8
P = 128           # partitions / tile edge
S = 1024          # species
NT = S // P       # species tiles (8)
B = 512           # batch
GRP = 4           # chunks per DMA group
GACT = 3          # chunks per Square group (3 PSUM banks)


def _balance_tiles(i2r0, i2r1, core_of, seconds=45.0, seed=0):
    """Simulated annealing on the species->tile assignment to minimize the
    total chunk count sum_bins ceil(max_core count / 128) over canonical
    (unordered) tile-pair bins."""
    import time as _time
    rng = np.random.default_rng(seed)
    tile = (np.arange(S) // P).astype(np.int64)
    adj = [[] for _ in range(S)]
    for i, (a, b) in enumerate(zip(i2r0, i2r1)):
        adj[a].append(i)
        if b != a:
            adj[b].append(i)
    adj = [np.array(x, np.int64) for x in adj]

    def full_M(tile):
        a = tile[i2r0]
        b = tile[i2r1]
        binid = np.minimum(a, b) * NT + np.maximum(a, b)
        M = np.zeros((NCORES, NT * NT), np.int64)
        for c in range(NCORES):
            M[c] = np.bincount(binid[core_of == c], minlength=NT * NT)
        return M

    M = full_M(tile)

    # chunk cost in matmuls: diagonal bins need 1 gather + 1 scatter,
    # off-diagonal need 2 gathers + 1 scatter
    wbin = np.array([2.0 if (b >> 3) == (b & 7) else 3.0
                     for b in range(NT * NT)])

    def obj(M):
        mx = M.max(axis=0)
        ch = np.ceil(mx / P)
        frac = np.where(mx > 0, (mx - 1) % P + 1, 0) / P
        return float((ch * wbin).sum() + 0.02 * frac.sum())

    cur = obj(M)
    Temp = 1.0
    t0 = _time.time()
    it = 0
    while _time.time() - t0 < seconds:
        it += 1
        s1, s2 = rng.integers(0, S, 2)
        if tile[s1] == tile[s2]:
            continue
        affected = np.unique(np.concatenate([adj[s1], adj[s2]]))
        old1, old2 = tile[s1], tile[s2]
        a = tile[i2r0[affected]]
        b = tile[i2r1[affected]]
        oldbins = np.minimum(a, b) * NT + np.maximum(a, b)
        tile[s1], tile[s2] = old2, old1
        a = tile[i2r0[affected]]
        b = tile[i2r1[affected]]
        newbins = np.minimum(a, b) * NT + np.maximum(a, b)
        cc = core_of[affected]
        np.subtract.at(M, (cc, oldbins), 1)
        np.add.at(M, (cc, newbins), 1)
        new = obj(M)
        if new <= cur or rng.random() < np.exp(-(new - cur) / Temp):
            cur = new
        else:
            np.subtract.at(M, (cc, newbins), 1)
            np.add.at(M, (cc, oldbins), 1)
            tile[s1], tile[s2] = old1, old2
        if it % 3000 == 0:
            Temp *= 0.75
    return tile


def _preprocess(y_in, i1r, i1p, r1, i2r0, i2r1, i2p, r2, sa_seconds=45.0):
    """Host-side index preprocessing. Returns per-core input dicts + schedule."""
    i1r = np.asarray(i1r).astype(np.int64)
    i1p = np.asarray(i1p).astype(np.int64)
    i2r0 = np.asarray(i2r0).astype(np.int64)
    i2r1 = np.asarray(i2r1).astype(np.int64)
    i2p = np.asarray(i2p).astype(np.int64)
    r1 = np.asarray(r1).astype(np.float32)
    r2 = np.asarray(r2).astype(np.float32)

    # ---- product -> core rebalancing (before the species SA, so the SA
    # optimizes against the final core assignment) ----
    # Chunk padding comes from max-over-cores of per-bin counts; assigning
    # products to cores greedily (vector scheduling on per-bin count
    # vectors) pushes each bin's max toward its mean. The host reassembles
    # output columns, so any product->core mapping is free.
    binid_all = (np.minimum(i2r0 >> 7, i2r1 >> 7) << 3) | \
        np.maximum(i2r0 >> 7, i2r1 >> 7)
    Vp = np.zeros((S, NT * NT), np.int64)
    np.add.at(Vp, (i2p, binid_all), 1)
    porder = np.argsort(-Vp.sum(axis=1), kind="stable")
    load = np.zeros((NCORES, NT * NT), np.int64)
    ccount = np.zeros(NCORES, np.int64)
    assign = np.empty(S, np.int64)
    curmax = np.zeros(NT * NT, np.int64)
    for p in porder:
        best, bestd = -1, None
        for c in range(NCORES):
            if ccount[c] >= P:
                continue
            d = np.maximum(load[c] + Vp[p] - curmax, 0).sum() * 1000 + ccount[c]
            if bestd is None or d < bestd:
                best, bestd = c, d
        assign[p] = best
        load[best] += Vp[p]
        ccount[best] += 1
        curmax = np.maximum(curmax, load[best])
    prod_cols = [np.where(assign == c)[0] for c in range(NCORES)]
    plocal = np.empty(S, np.int64)
    for c in range(NCORES):
        plocal[prod_cols[c]] = np.arange(P)
    core_of = assign[i2p]

    # Species permutation: SA-balanced tile assignment, then renumber
    # species so each tile is contiguous.
    tile_of = _balance_tiles(i2r0, i2r1, core_of, seconds=sa_seconds)
    order = np.argsort(tile_of, kind="stable")  # new_idx -> orig species
    new_idx = np.empty(S, np.int64)
    new_idx[order] = np.arange(S)

    i1r = new_idx[i1r]
    i2r0 = new_idx[i2r0]
    i2r1 = new_idx[i2r1]

    # Canonicalize second-order pairs so tile(r0) <= tile(r1)
    swap = (i2r0 >> 7) > (i2r1 >> 7)
    i2r0s = np.where(swap, i2r1, i2r0)
    i2r1s = np.where(swap, i2r0, i2r1)
    i2r0, i2r1 = i2r0s, i2r1s

    # Dense first-order matrix W1[s, p] = sum of rates (permuted rows)
    W1 = np.zeros((S, S), np.float32)
    np.add.at(W1, (i1r, i1p), r1)
    # Dense second-order correction W2[s, p] = -sum rate/2 over reactions
    # where s appears as either reactant and p is the product.
    W2 = np.zeros((S, S), np.float32)
    np.add.at(W2, (i2r0, i2p), -0.5 * r2)
    np.add.at(W2, (i2r1, i2p), -0.5 * r2)

    yT = np.ascontiguousarray(np.asarray(y_in, np.float32).T[order])  # [S, B] permuted

    g2 = np.sqrt(0.5 * r2).astype(np.float32)

    T0 = i2r0 >> 7
    T1 = i2r1 >> 7
    binid = (T0 << 3) | T1  # canonical: T0 <= T1

    counts = np.zeros((NCORES, NT * NT), np.int64)
    for c in range(NCORES):
        counts[c] = np.bincount(binid[core_of == c], minlength=NT * NT)
    maxc = counts.max(axis=0)

    # ---- chunk layout: full per-bin chunks + remainder-merged chunks ----
    # Each chunk descriptor: tiles = [gather tiles in matmul order],
    # mat_of_tile = {tile: matrix position within chunk},
    # spans = [(bin, col0, ncols)]
    full_n = maxc // P
    rem_n = maxc % P
    chunk_descs = []
    bin_slots = {}  # bin -> list of (chunk_idx, col0) per 128-block, in order
    for b in range(NT * NT):
        if maxc[b] == 0:
            continue
        t0, t1 = b >> 3, b & 7
        slots = []
        for _f in range(int(full_n[b])):
            tiles = [t0] if t0 == t1 else [t0, t1]
            chunk_descs.append(dict(tiles=tiles,
                                    mat_of_tile={t: i for i, t in enumerate(tiles)},
                                    spans=[(b, 0, P)]))
            slots.append((len(chunk_descs) - 1, 0))
        bin_slots[b] = slots

    # remainders: greedy pack by shared tile
    rem = {b: int(rem_n[b]) for b in range(NT * NT) if rem_n[b] > 0}
    while rem:
        # pick the tile with the largest total remaining mass
        mass = np.zeros(NT, np.int64)
        for b, sz in rem.items():
            t0, t1 = b >> 3, b & 7
            mass[t0] += sz
            if t1 != t0:
                mass[t1] += sz
        t = int(mass.argmax())
        members = sorted((b for b in rem if (b >> 3) == t or (b & 7) == t),
                         key=lambda b: -rem[b])
        take, cap = [], P
        for b in members:
            if rem[b] <= cap:
                take.append(b)
                cap -= rem[b]
        if not take:
            take = [members[0]]  # oversized cannot happen (rem < P) but guard
        tiles = [t]
        mat_of_tile = {t: 0}
        spans = []
        col = 0
        for b in take:
            t0, t1 = b >> 3, b & 7
            other = t1 if t0 == t else t0
            if other != t and other not in mat_of_tile:
                mat_of_tile[other] = len(tiles)
                tiles.append(other)
            spans.append((b, col, rem[b]))
            bin_slots.setdefault(b, []).append((len(chunk_descs), col))
            col += rem[b]
            del rem[b]
        chunk_descs.append(dict(tiles=tiles, mat_of_tile=mat_of_tile, spans=spans))

    nchunk = len(chunk_descs)
    # gather-matrix stream positions
    mat_base = []
    nmat = 0
    for d in chunk_descs:
        mat_base.append(nmat)
        nmat += len(d["tiles"])
    ngmat = math.ceil(nmat / GRP)
    nmatpad = ngmat * GRP
    ngroup = math.ceil(nchunk / GRP)
    nchpad = ngroup * GRP

    sched = [list(d["tiles"]) for d in chunk_descs]

    # per-reaction (chunk, col, mats) mapping is bin-local; build per core
    in_maps = []
    for c in range(NCORES):
        sel = core_of == c
        bsel = binid[sel]
        order = np.argsort(bsel, kind="stable")
        bs = bsel[order]
        r0a = i2r0[sel][order]
        r1a = i2r1[sel][order]
        pl = plocal[i2p[sel]][order]
        gg = g2[sel][order]
        bin_start = np.zeros(NT * NT, np.int64)
        cnt = np.bincount(bs, minlength=NT * NT)
        bin_start[1:] = np.cumsum(cnt)[:-1]
        pos = np.arange(len(bs)) - bin_start[bs]

        # chunk + col for each reaction, from its bin slot table
        chunk = np.empty(len(bs), np.int64)
        col = np.empty(len(bs), np.int64)
        for b in np.unique(bs):
            slots = bin_slots[b]
            m = bs == b
            p_ = pos[m]
            blk = (p_ >> 7).astype(np.int64)
            ci = np.array([slots[k][0] for k in blk])
            c0 = np.array([slots[k][1] for k in blk])
            chunk[m] = ci
            col[m] = c0 + (p_ & 127)

        GS = np.zeros((nmatpad, P, P), np.float32)
        SC = np.zeros((nchpad, P, P), np.float32)
        # place gather entries: r0 into its tile's matrix, r1 into its
        mat0 = np.empty(len(bs), np.int64)
        mat1 = np.empty(len(bs), np.int64)
        for i in range(len(bs)):
            d = chunk_descs[chunk[i]]
            base = mat_base[chunk[i]]
            mat0[i] = base + d["mat_of_tile"][r0a[i] >> 7]
            mat1[i] = base + d["mat_of_tile"][r1a[i] >> 7]
        np.add.at(GS, (mat0, r0a & 127, col), gg)
        np.add.at(GS, (mat1, r1a & 127, col), gg)
        SC[chunk, col, pl] = 1.0

        def grp(x, ng):
            return np.ascontiguousarray(
                x.reshape(ng, GRP, P, P).transpose(0, 2, 1, 3)
                .reshape(ng, P, GRP * P).astype(np.float16)
            )

        in_maps.append(
            dict(
                yT=yT.astype(np.float16),
                W1g=np.ascontiguousarray(W1[:, prod_cols[c]]).astype(np.float16),
                W2g=np.ascontiguousarray(W2[:, prod_cols[c]]).astype(np.float16),
                GS=grp(GS, ngmat),
                SCT=grp(SC, ngroup),
            )
        )
    return in_maps, sched, nchunk, (ngroup, ngmat, prod_cols)


def _build(nchunk, ngroups, sched, reps=1, bufs_oh=6, bufs_u=2, bufs_u2=4,
           bufs_acc=2, sq_pattern="A"):
    """sq_pattern: cycle of engines for the Square groups ('A'=ACT, 'D'=DVE)."""
    ngroup, ngmat = ngroups[0], ngroups[1]
    nc = bacc.Bacc("TRN2", target_bir_lowering=False, debug=False, num_devices=NCORES)

    yT_d = nc.dram_tensor("yT", [S, B], F16, kind="ExternalInput").ap()
    w1_d = nc.dram_tensor("W1g", [S, P], F16, kind="ExternalInput").ap()
    w2_d = nc.dram_tensor("W2g", [S, P], F16, kind="ExternalInput").ap()
    gs_d = nc.dram_tensor("GS", [ngmat, P, GRP * P], F16, kind="ExternalInput").ap()
    sc_d = nc.dram_tensor("SCT", [ngroup, P, GRP * P], F16, kind="ExternalInput").ap()
    out_d = nc.dram_tensor("out", [P, B], F32, kind="ExternalOutput").ap()

    nact = math.ceil(nchunk / GACT)

    with tile.TileContext(nc) as tc:
        with (
            tc.tile_pool(name="res", bufs=1) as res,
            tc.tile_pool(name="oh", bufs=bufs_oh) as ohp,
            tc.tile_pool(name="work", bufs=3) as wp,
            tc.tile_pool(name="u2p", bufs=bufs_u2) as u2p,
            tc.tile_pool(name="acc", bufs=bufs_acc, space="PSUM") as accp,
            tc.tile_pool(name="up", bufs=bufs_u, space="PSUM") as upp,
        ):
            # Resident tiles: y^T species tiles, W1/W2 slices
            yts = []
            for t in range(NT):
                yt = res.tile([P, B], F16, tag=f"yt{t}")
                nc.sync.dma_start(yt[:], yT_d[t * P:(t + 1) * P, :])
                yts.append(yt)
            w1t = res.tile([P, NT * P], F16, tag="w1")
            w2t = res.tile([P, NT * P], F16, tag="w2")
            for t in range(NT):
                nc.sync.dma_start(w1t[:, t * P:(t + 1) * P], w1_d[t * P:(t + 1) * P, :])
                nc.sync.dma_start(w2t[:, t * P:(t + 1) * P], w2_d[t * P:(t + 1) * P, :])

            def one_pass():
                acc = accp.tile([P, B], F32, space="PSUM", tag="acc")

                # w = y^2 per species tile (DVE, fp16 2x) — needed only by
                # the trailing dense W2 matmuls, so plenty of slack.
                wts = []
                for t in range(NT):
                    w = wp.tile([P, B], F16, tag=f"w{t}")
                    nc.vector.tensor_tensor(out=w[:], in0=yts[t][:], in1=yts[t][:],
                                            op=mybir.AluOpType.mult)
                    wts.append(w)

                # First-order dense matmuls open the accumulation group.
                for t in range(NT):
                    nc.tensor.matmul(acc[:], lhsT=w1t[:, t * P:(t + 1) * P],
                                     rhs=yts[t][:], start=(t == 0), stop=False)

                # Second-order chunks, grouped by GACT for the Square and by
                # GRP for DMA. Scatters lag one Square-group behind gathers.
                from collections import deque
                pendq = deque()  # (u2_tile, [(chunk_k, sc_slice)...]) with lag 2
                gi_cur = -1
                mi_cur = -1
                mpos = 0  # position in the gather-matrix stream
                gsg = None
                for ga in range(nact):
                    c0 = ga * GACT
                    cw = min(GACT, nchunk - c0)
                    ug = upp.tile([P, GACT * B], F32, space="PSUM", tag="ug")
                    scs = []
                    pend = pendq.popleft() if len(pendq) >= 2 else None
                    psc = list(pend[1]) if pend is not None else []
                    for k in range(cw):
                        c = c0 + k
                        tiles = sched[c]
                        gi = c // GRP
                        if gi != gi_cur:
                            gi_cur = gi
                            scg = ohp.tile([P, GRP * P], F16, tag="scg")
                            nc.sync.dma_start(scg[:], sc_d[gi])
                        us = ug[:, k * B:(k + 1) * B]
                        nmt = len(tiles)
                        for mi, t in enumerate(tiles):
                            mg, mk = divmod(mpos, GRP)
                            if mg != mi_cur:
                                mi_cur = mg
                                gsg = ohp.tile([P, GRP * P], F16, tag="gsg")
                                nc.sync.dma_start(gsg[:], gs_d[mg])
                            mpos += 1
                            nc.tensor.matmul(us, lhsT=gsg[:, mk * P:(mk + 1) * P],
                                             rhs=yts[t][:],
                                             start=(mi == 0), stop=(mi == nmt - 1))
                        cs = slice((c % GRP) * P, (c % GRP + 1) * P)
                        scs.append((k, scg[:, cs]))
                        # interleave one pending scatter after each gather pair
                        if psc:
                            pk, pscsl = psc.pop(0)
                            nc.tensor.matmul(acc[:], lhsT=pscsl,
                                             rhs=pend[0][:, pk * B:(pk + 1) * B],
                                             start=False, stop=False)

                    u2 = u2p.tile([P, GACT * B], F16, tag="u2")
                    eng = sq_pattern[ga % len(sq_pattern)]
                    if eng == "A":
                        nc.scalar.activation(u2[:, :cw * B], ug[:, :cw * B],
                                             mybir.ActivationFunctionType.Square)
                    else:
                        nc.vector.tensor_tensor(out=u2[:, :cw * B],
                                                in0=ug[:, :cw * B],
                                                in1=ug[:, :cw * B],
                                                op=mybir.AluOpType.mult)

                    # drain any leftover pending scatters
                    for pk, pscsl in psc:
                        nc.tensor.matmul(acc[:], lhsT=pscsl,
                                         rhs=pend[0][:, pk * B:(pk + 1) * B],
                                         start=False, stop=False)
                    pendq.append((u2, scs))

                while pendq:
                    pu2, pscs = pendq.popleft()
                    for k, scsl in pscs:
                        nc.tensor.matmul(acc[:], lhsT=scsl,
                                         rhs=pu2[:, k * B:(k + 1) * B],
                                         start=False, stop=False)

                # Dense W2 correction on w closes the accumulation group.
                for t in range(NT):
                    nc.tensor.matmul(acc[:], lhsT=w2t[:, t * P:(t + 1) * P],
                                     rhs=wts[t][:], start=False, stop=(t == NT - 1))

                outs = wp.tile([P, B], F32, tag="outs")
                nc.vector.tensor_copy(outs[:], acc[:])
                nc.sync.dma_start(out_d[:], outs[:])

            for _rep in range(reps):
                one_pass()

    nc.compile()
    return nc


def _run(inputs, trace=False):
    in_maps, sched, nchunk, ngroup = _preprocess(
        inputs["y_in"], inputs["inds_1r"], inputs["inds_1p"], inputs["rate_1"],
        inputs["inds_2r0"], inputs["inds_2r1"], inputs["inds_2p"], inputs["rate_2"],
    )
    nc = _build(nchunk, ngroup, sched)
    res = None
    y_out = None
    last_exc = None
    for attempt in range(3):
        try:
            res = run_bass_kernel_spmd(nc, in_maps, list(range(NCORES)), trace=trace)
        except Exception as e:  # transient device wedges (NRT_EXEC_UNIT_...)
            last_exc = e
            import time as _time
            _time.sleep(2.0)
            continue
        y_out = np.empty((B, S), np.float32)
        for c in range(NCORES):
            y_out[:, ngroup[2][c]] = res.results[c]["out"].T
        # guard against silent corruption from a wedged device
        if np.isfinite(y_out).all() and not (y_out == 0).all():
            break
        y_out = None
    if y_out is None:
        if last_exc is not None:
            raise last_exc
        raise RuntimeError("kernel produced non-finite/empty output on all attempts")
    return y_out, res


def kernel(**inputs) -> np.ndarray:
    return _run(inputs, trace=False)[0]
